# revision 17
# baseline (speedup 1.0000x reference)
"""DCE-modulated ResBlock (dense_cnn) on 8 Trainium2 NeuronCores.

Data-parallel over batch (16 images -> 2 per core), weights replicated.
BatchNorm batch statistics are made exact via two cross-core AllReduces
(sync-BN): one for bn1, one for bn2+shortcut-bn.

v3 vs baseline:
- shortcut conv runs entirely BEFORE any AllReduce-dependent vector op:
  in-order engine streams mean ar1_post's bn_affine head-of-line blocked
  the DVE bnstats the shortcut needed, so the PE idled through the whole
  AllReduce wait in the old ordering.
- conv2/shortcut outputs are cached to SBUF in bf16 by the ACT engine
  during the stats pass; the final pass is then pure DVE/ACT affine+silu
  with no recomputation matmuls. t1/w2 are bf16 (same PE rate, half SBUF
  and DMA); x/xm/w1/wsc stay f32r because DVE/Pool *elementwise* on bf16
  measured 2.5-20x slower than f32.
- bn_affine's rsqrt runs on DVE (pow -0.5): keeps Sqrt off the ACT table
  cache so Silu/Sigmoid never reload mid-critical-path; a dummy Silu op
  pre-warms the table during the AllReduce wait.
- final pass is an ACT-led 2-op pipeline with the DVE add in between.
"""

from contextlib import ExitStack

import numpy as np
import ml_dtypes

import concourse.bass as bass
import concourse.mybir as mybir
from concourse import tile
from concourse.bass_utils import run_bass_kernel_spmd

F32 = mybir.dt.float32
F32R = mybir.dt.float32r
BF16 = mybir.dt.bfloat16
AF = mybir.ActivationFunctionType
ALU = mybir.AluOpType

B, C, H, W = 16, 256, 64, 64
LDCE, CDCE = 100, 128
NCORES = 8
NB = B // NCORES          # images per core
MT = C // 128             # channel tiles (2)
PW = W + 2                # padded row width 66
PLEN = (H + 2) * PW + 2   # padded buffer + 2 guard cols (4358)
RG = 8                    # row groups per image
RGR = H // RG             # rows per group (8)
TLEN = RGR * W            # columns per psum tile (512)
TSP = RGR * PW            # padded columns spanned per group (528)
NLOC = NB * H * W         # local reduction count per channel (8192)
NGLB = B * H * W          # global reduction count (65536)
EPS = 1e-5
XSPLIT = 67 + 32 * PW     # 2179: row 0-31 sums need only the first chunk


def _split_sync_waits(nc, max_waits=1):
    """This container's walrus build accepts only one sync-wait command per
    instruction; hoist excess waits onto same-engine NoOps placed before."""
    for f in nc.m.functions:
        for bb in f.blocks:
            insts = bb.instructions
            if not any(
                i.sync_info is not None and len(i.sync_info.on_wait) > max_waits
                for i in insts
            ):
                continue
            newlist = []
            for inst in insts:
                si = inst.sync_info
                if si is not None and len(si.on_wait) > max_waits:
                    waits = list(si.on_wait)
                    extra, keep = waits[:-max_waits], waits[-max_waits:]
                    for j in range(0, len(extra), max_waits):
                        nop = mybir.InstNoOp(name=f"{inst.name}-sw{j}", ins=[], outs=[])
                        nop.engine = inst.engine
                        nop.sync_info = mybir.SyncInfo(
                            on_wait=extra[j : j + max_waits], on_update=[]
                        )
                        newlist.append(nop)
                    inst.sync_info = mybir.SyncInfo(
                        on_wait=keep, on_update=list(si.on_update)
                    )
                newlist.append(inst)
            bb.instructions = newlist


def _bn_stats_raw(nc, out_ap, in_ap):
    """One HW BNStats chunk (count/mean/count*var for even+odd lanes) over
    the full (possibly strided) input AP; bass's shape assert only allows
    flat inputs, walrus only allows 6 outputs, so emit the IR directly."""
    eng = nc.vector
    eng.add_instruction(
        mybir.InstBNStats(
            name=nc.get_next_instruction_name(),
            ins=[eng.lower_ap(in_ap)],
            outs=[eng.lower_ap(out_ap)],
        )
    )


def _build():
    nc = bass.Bass(
        "TRN2",
        target_bir_lowering=False,
        debug=False,
        num_devices=NCORES,
        use_seq_codegen=True,
        num_swdge_queues=4,
    )

    # ---- kernel I/O (per-core shapes) ----
    xp_d = nc.dram_tensor("xp", [NB, C, PLEN], BF16, kind="ExternalInput")
    dce_d = nc.dram_tensor("dce", [NB, LDCE, CDCE], F32, kind="ExternalInput")
    w1t_d = nc.dram_tensor("w1t", [9, C, C], BF16, kind="ExternalInput")
    w2t_d = nc.dram_tensor("w2t", [C, C], BF16, kind="ExternalInput")
    wsct_d = nc.dram_tensor("wsct", [C, C], BF16, kind="ExternalInput")
    wdce_d = nc.dram_tensor("wdce_t", [CDCE, C], F32, kind="ExternalInput")
    wst_d = nc.dram_tensor("wst", [C, C // 2], F32, kind="ExternalInput")
    wet_d = nc.dram_tensor("wet", [C // 2, C], F32, kind="ExternalInput")
    chc_d = nc.dram_tensor("chc", [C, 9], F32, kind="ExternalInput")
    # per-channel vectors: [b_dce, g1, be1, g2, be2, gs, bes, b_expand]
    chv_d = nc.dram_tensor("chv", [C, 12], F32, kind="ExternalInput")
    bsh_d = nc.dram_tensor("bsh", [C // 2], F32, kind="ExternalInput")
    y_d = nc.dram_tensor("y", [NB, C, H, W], F32, kind="ExternalOutput")

    # collective bounce buffers (one pair per AllReduce so they pipeline)
    cc1_in = {mt: nc.dram_tensor(f"cc1_in{mt}", [128, 2], F32) for mt in range(MT)}
    cc1_out = {
        mt: nc.dram_tensor(f"cc1_out{mt}", [128, 2], F32, addr_space="Shared")
        for mt in range(MT)
    }
    cc2_in = {mt: nc.dram_tensor(f"cc2_in{mt}", [128, 4], F32) for mt in range(MT)}
    cc2_out = {
        mt: nc.dram_tensor(f"cc2_out{mt}", [128, 4], F32, addr_space="Shared")
        for mt in range(MT)
    }
    groups = [list(range(NCORES))]

    with tile.TileContext(nc) as tc, ExitStack() as es:
        pers = es.enter_context(tc.tile_pool(name="pers", bufs=1))
        stage = es.enter_context(tc.tile_pool(name="stage", bufs=4))

        # ---- persistent SBUF buffers ----
        xm = {}   # padded x, later x*mod (f32r)
        t1 = {}   # conv1 out, later silu(bn1(.)) (bf16)
        for b in range(NB):
            for ct in range(MT):
                xm[b, ct] = pers.tile([128, PLEN], BF16, tag=f"xm{b}{ct}", name=f"xm{b}{ct}")
                t1[b, ct] = pers.tile([128, H * W], BF16, tag=f"t1{b}{ct}", name=f"t1{b}{ct}")

        # ---- ACT table preloads (hidden under the x DMA) ----
        dummy = pers.tile([128, 1], F32, tag="dummy", name="dummy")
        nc.vector.memset(dummy[:], 1.0)
        nc.scalar.activation(dummy[:], dummy[:], AF.Sigmoid)

        # ---- load x first (x gates the whole pipeline); 4 queues ----
        xq = {(0, 0): (nc.sync, nc.sync), (0, 1): (nc.scalar, nc.scalar),
              (1, 0): (nc.gpsimd, nc.gpsimd), (1, 1): (nc.sync, nc.scalar)}
        for b in range(NB):
            for ct in range(MT):
                e1, e2 = xq[b, ct]
                e1.dma_start(
                    xm[b, ct][:, 0:XSPLIT],
                    xp_d[b, ct * 128 : ct * 128 + 128, 0:XSPLIT],
                )
                e2.dma_start(
                    xm[b, ct][:, XSPLIT:PLEN],
                    xp_d[b, ct * 128 : ct * 128 + 128, XSPLIT:PLEN],
                )

        w1 = {}
        for mt in range(MT):      # mt outer: conv1(mt0) weights land first
            for kt in range(MT):
                big = pers.tile([128, 9 * 128], BF16, tag=f"w1b{kt}{mt}", name=f"w1b{kt}{mt}")
                nc.gpsimd.dma_start(
                    big[:].rearrange("p (t o) -> p t o", t=9),
                    w1t_d[:, kt * 128 : kt * 128 + 128, mt * 128 : mt * 128 + 128]
                    .rearrange("t c o -> c t o"),
                )
                for tap in range(9):
                    w1[tap, kt, mt] = big[:, tap * 128 : (tap + 1) * 128]
        w2 = {}
        wsc = {}
        for kt in range(MT):
            bw = pers.tile([128, 2 * 128], BF16, tag=f"w2b{kt}", name=f"w2b{kt}")
            nc.sync.dma_start(
                bw[:].rearrange("p (m o) -> p m o", m=MT),
                w2t_d[kt * 128 : kt * 128 + 128, :].rearrange("c (m o) -> c m o", m=MT),
            )
            bs = pers.tile([128, 2 * 128], BF16, tag=f"wscb{kt}", name=f"wscb{kt}")
            nc.sync.dma_start(
                bs[:].rearrange("p (m o) -> p m o", m=MT),
                wsct_d[kt * 128 : kt * 128 + 128, :].rearrange("c (m o) -> c m o", m=MT),
            )
            for mt in range(MT):
                w2[kt, mt] = bw[:, mt * 128 : (mt + 1) * 128]
                wsc[kt, mt] = bs[:, mt * 128 : (mt + 1) * 128]
        wdce = {}
        wet = {}
        chv = {}
        chc = {}
        for mt in range(MT):
            wdce[mt] = pers.tile([128, 128], F32, tag=f"wdce{mt}", name=f"wdce{mt}")
            nc.sync.dma_start(wdce[mt][:], wdce_d[:, mt * 128 : mt * 128 + 128])
            wet[mt] = pers.tile([128, 128], F32, tag=f"wet{mt}", name=f"wet{mt}")
            nc.sync.dma_start(wet[mt][:], wet_d[:, mt * 128 : mt * 128 + 128])
            chv[mt] = pers.tile([128, 12], F32, tag=f"chv{mt}", name=f"chv{mt}")
            nc.sync.dma_start(chv[mt][:], chv_d[mt * 128 : mt * 128 + 128, :])
            chc[mt] = pers.tile([128, 9], F32, tag=f"chc{mt}", name=f"chc{mt}")
            nc.sync.dma_start(chc[mt][:], chc_d[mt * 128 : mt * 128 + 128, :])
        wst = {}
        for kt in range(MT):
            wst[kt] = pers.tile([128, 128], F32, tag=f"wst{kt}", name=f"wst{kt}")
            nc.sync.dma_start(wst[kt][:], wst_d[kt * 128 : kt * 128 + 128, :])
        bsh = pers.tile([128, 1], F32, tag="bsh", name="bsh")
        nc.sync.dma_start(bsh[:], bsh_d[:].rearrange("(p a) -> p a", a=1))

        # =====================================================================
        # Phase A: modulation gate
        # =====================================================================
        esA = ExitStack()
        psA = esA.enter_context(tc.tile_pool(name="psA", bufs=2, space="PSUM"))

        ones_f = pers.tile([128, 1], F32, tag="ones_f", name="ones_f")
        nc.vector.memset(ones_f[:], 1.0)
        ones_row = pers.tile([1, 128], F32, tag="ones_row", name="ones_row")
        nc.vector.memset(ones_row[:], 1.0)
        idn = pers.tile([128, 128], F32, tag="idn", name="idn")
        iot = pers.tile([128, 128], mybir.dt.int32, tag="iot", name="iot")
        nc.gpsimd.iota(iot[:], [[1, 128]], channel_multiplier=-1)
        nc.vector.tensor_scalar(idn[:], iot[:], 0, None, op0=ALU.is_equal)

        pooled = pers.tile([128, NB], F32, tag="pooled", name="pooled")  # dce mean, CDCE x img
        for b in range(NB):
            dce_sb = stage.tile([LDCE, CDCE], F32, tag="dce_sb", name="dce_sb")
            nc.sync.dma_start(dce_sb[:], dce_d[b, :, :])
            ps = psA.tile([128, 1], F32, tag="tiny", name="tiny")
            nc.tensor.matmul(ps[:], dce_sb[:], ones_f[0:LDCE, :], start=True, stop=True)
            nc.scalar.mul(pooled[:, b : b + 1], ps[:], 1.0 / LDCE)

        # spatial_proj via border-sum identity; gath cols:
        # [S, rowE, row0, colE, col0, x(E,E), x(E,0), x(0,E), x(0,0)]
        # row sums include the zero padding cols, so one [64,66] reduce works;
        # rows 0-31 need only the first x chunk, rows 32-63 go on Pool.
        sp = {}
        for ct in range(MT):
            sp[ct] = pers.tile([128, NB], F32, tag=f"sp{ct}", name=f"sp{ct}")
        for b in range(NB):
            for ct in range(MT):
                buf = xm[b, ct]
                gath = stage.tile([128, 9], F32, tag="gath", name="gath")
                rows = stage.tile([128, H], F32, tag="rows", name="rows")
                halfA = buf[:, 66 : 66 + 32 * PW].rearrange("p (r c) -> p r c", r=32)
                halfB = buf[:, 66 + 32 * PW : 66 + 64 * PW].rearrange(
                    "p (r c) -> p r c", r=32
                )
                nc.vector.reduce_sum(rows[:, 0:32], halfA, axis=mybir.AxisListType.X)
                nc.vector.reduce_sum(rows[:, 32:64], halfB, axis=mybir.AxisListType.X)
                nc.vector.reduce_sum(gath[:, 0:1], rows[:], axis=mybir.AxisListType.X)
                nc.vector.tensor_copy(gath[:, 1:2], rows[:, H - 1 : H])
                nc.vector.tensor_copy(gath[:, 2:3], rows[:, 0:1])
                colE = buf[:, 67 + W - 1 : 67 + W - 1 + H * PW].rearrange(
                    "p (r c) -> p r c", r=H
                )[:, :, 0:1]
                col0 = buf[:, 67 : 67 + H * PW].rearrange(
                    "p (r c) -> p r c", r=H
                )[:, :, 0:1]
                nc.vector.reduce_sum(gath[:, 3:4], colE, axis=mybir.AxisListType.XY)
                nc.vector.reduce_sum(gath[:, 4:5], col0, axis=mybir.AxisListType.XY)
                be = 67 + (H - 1) * PW
                nc.vector.tensor_copy(gath[:, 5:6], buf[:, be + W - 1 : be + W])
                nc.vector.tensor_copy(gath[:, 6:7], buf[:, be : be + 1])
                nc.vector.tensor_copy(gath[:, 7:8], buf[:, 67 + W - 1 : 67 + W])
                nc.vector.tensor_copy(gath[:, 8:9], buf[:, 67 : 68])
                gm = stage.tile([128, 9], F32, tag="gm", name="gm")
                nc.vector.tensor_tensor(gm[:], gath[:], chc[ct][:], op=ALU.mult)
                nc.vector.reduce_sum(
                    sp[ct][:, b : b + 1], gm[:], axis=mybir.AxisListType.X
                )

        # m = (dce_pooled @ w_dce.T + b_dce) * spatial_proj   -> [128, NB]/ct
        m_r = {}
        for mt in range(MT):
            ps = psA.tile([128, NB], F32, tag="tiny", name="tiny")
            nc.tensor.matmul(ps[:], wdce[mt][:], pooled[:], start=True, stop=True)
            dcep = stage.tile([128, NB], F32, tag="dcep", name="dcep")
            nc.scalar.add(dcep[:], ps[:], chv[mt][:, 0:1])
            m_r[mt] = pers.tile([128, NB], F32, tag=f"m{mt}", name=f"m{mt}")
            nc.vector.tensor_tensor(m_r[mt][:], dcep[:], sp[mt][:], op=ALU.mult)

        # h = relu(m @ w_shrink.T + b_shrink)  -> [128, NB]
        ps_h = psA.tile([128, NB], F32, tag="tiny", name="tiny")
        for kt in range(MT):
            nc.tensor.matmul(
                ps_h[:], wst[kt][:], m_r[kt][:], start=(kt == 0), stop=(kt == MT - 1)
            )
        h_r = pers.tile([128, NB], F32, tag="h_r", name="h_r")
        nc.scalar.activation(h_r[:], ps_h[:], AF.Relu, bias=bsh[:])

        # mod = sigmoid(h @ w_expand.T + b_expand)  -> f32 [128, NB] per ct
        mod = {}
        for mt in range(MT):
            ps = psA.tile([128, NB], F32, tag="tiny", name="tiny")
            nc.tensor.matmul(ps[:], wet[mt][:], h_r[:], start=True, stop=True)
            mod[mt] = pers.tile([128, NB], F32, tag=f"mod{mt}", name=f"mod{mt}")
            nc.scalar.activation(mod[mt][:], ps[:], AF.Sigmoid, bias=chv[mt][:, 7:8])

        # xm = x * mod (in place, chunked so conv1's first windows unblock
        # early; chunks split over DVE/Pool/ACT)
        XCH = [0, 1190, 2180, 3270, PLEN]
        for i in range(len(XCH) - 1):
            for b in range(NB):
                for ct in range(MT):
                    s, e = XCH[i], XCH[i + 1]
                    nc.scalar.activation(
                        xm[b, ct][:, s:e], xm[b, ct][:, s:e],
                        AF.Copy, scale=mod[ct][:, b : b + 1],
                    )
        esA.close()

        # =====================================================================
        # Phase B: conv1 (+bn1 stats); shortcut conv fully before any
        # AllReduce-dependent op so the PE fills the AllReduce wait.
        # =====================================================================
        bnb1 = {mt: pers.tile([128, NB * RG, 6], F32, tag=f"bnb1{mt}", name=f"bnb1{mt}") for mt in range(MT)}
        bnbs = {mt: pers.tile([128, NB * RG, 6], F32, tag=f"bnbs{mt}", name=f"bnbs{mt}") for mt in range(MT)}
        bnb2 = {mt: pers.tile([128, NB * RG, 6], F32, tag=f"bnb2{mt}", name=f"bnb2{mt}") for mt in range(MT)}

        taps = [((kh - 1) * PW + (kw - 1), 3 * kh + kw) for kh in range(3) for kw in range(3)]

        esB = ExitStack()
        psB = esB.enter_context(tc.tile_pool(name="psB", bufs=2, space="PSUM"))

        def win(buf, rg, off=0):
            s = 67 + rg * TSP + off
            return buf[:, s : s + RGR * PW].rearrange("p (r c) -> p r c", r=RGR)[
                :, :, 0:W
            ]

        def conv1_mt(mt):
            for b in range(NB):
                for rg in range(RG):
                    ps = psB.tile([128, TLEN], F32, tag="c1", name="c1", bufs=2)
                    first = True
                    for kt in range(MT):
                        for off, tap in taps:
                            nc.tensor.matmul(
                                ps[:],
                                w1[tap, kt, mt],
                                win(xm[b, kt], rg, off),
                                start=first,
                                stop=(kt == MT - 1 and tap == 8),
                            )
                            first = False
                    _bn_stats_raw(nc, bnb1[mt][:, b * RG + rg, :], ps[:])
                    nc.scalar.copy(
                        t1[b, mt][:, rg * TLEN : (rg + 1) * TLEN], ps[:]
                    )

        # local chunk stats -> (sum, sum_x2) packed for the allreduce
        def local_sums(bnb, mt, dst_sum, dst_ex2):
            mv = stage.tile([128, 2], F32, tag="mv", name="mv")
            nc.vector.bn_aggr(
                mv[:],
                bnb[mt][:]
                .rearrange("p a s -> p (a s)")
                .rearrange("p (a b) -> p a b", b=3),
            )
            nc.vector.tensor_scalar_mul(dst_sum, mv[:, 0:1], float(NLOC))
            t = stage.tile([128, 1], F32, tag="tloc", name="tloc")
            nc.vector.tensor_tensor(t[:], mv[:, 0:1], mv[:, 0:1], op=ALU.mult)
            nc.vector.tensor_tensor(t[:], t[:], mv[:, 1:2], op=ALU.add)
            nc.vector.tensor_scalar_mul(dst_ex2, t[:], float(NLOC))

        # global bn affine: a = g*rsqrt(var+eps), c = be - mean*a
        def bn_affine(sum_ap, ex2_ap, g_ap, be_ap, a_dst, c_dst, wdt=1):
            mean = stage.tile([128, wdt], F32, tag=f"bnm{wdt}", name="bnm")
            nc.vector.tensor_scalar_mul(mean[:], sum_ap, 1.0 / NGLB)
            var = stage.tile([128, wdt], F32, tag=f"bnv{wdt}", name="bnv")
            nc.vector.tensor_scalar_mul(var[:], ex2_ap, 1.0 / NGLB)
            t = stage.tile([128, wdt], F32, tag=f"bnt{wdt}", name="bnt")
            nc.vector.tensor_tensor(t[:], mean[:], mean[:], op=ALU.mult)
            nc.vector.tensor_tensor(var[:], var[:], t[:], op=ALU.subtract)
            nc.vector.tensor_scalar_add(var[:], var[:], EPS)
            # rsqrt on DVE (bit-trick + 2 Newton steps) so the ACT table
            # cache never has to swap Silu out for Sqrt mid-critical-path
            yq = stage.tile([128, wdt], F32, tag=f"yq{wdt}", name="yq")
            nc.vector.tensor_scalar(
                yq[:].bitcast(mybir.dt.int32), var[:].bitcast(mybir.dt.int32),
                1, None, op0=ALU.logical_shift_right,
            )
            nc.vector.tensor_scalar(
                yq[:].bitcast(mybir.dt.int32), yq[:].bitcast(mybir.dt.int32),
                -1, 0x5F3759DF, op0=ALU.mult, op1=ALU.add,
            )
            h = stage.tile([128, wdt], F32, tag=f"hq{wdt}", name="hq")
            nc.vector.tensor_scalar_mul(h[:], var[:], 0.5)
            t2 = stage.tile([128, wdt], F32, tag=f"t2q{wdt}", name="t2q")
            for _ in range(2):
                nc.vector.tensor_tensor(t2[:], yq[:], yq[:], op=ALU.mult)
                nc.vector.tensor_tensor(t2[:], t2[:], h[:], op=ALU.mult)
                nc.vector.tensor_scalar(
                    t2[:], t2[:], -1.0, 1.5, op0=ALU.mult, op1=ALU.add
                )
                nc.vector.tensor_tensor(yq[:], yq[:], t2[:], op=ALU.mult)
            nc.vector.tensor_tensor(a_dst, yq[:], g_ap, op=ALU.mult)
            nc.vector.tensor_tensor(t[:], mean[:], a_dst, op=ALU.mult)
            nc.vector.tensor_tensor(c_dst, be_ap, t[:], op=ALU.subtract)

        ar1 = {mt: pers.tile([128, 2], F32, tag=f"ar1{mt}", name=f"ar1{mt}") for mt in range(MT)}
        g1s = {mt: pers.tile([128, 2], F32, tag=f"g1s{mt}", name=f"g1s{mt}") for mt in range(MT)}
        a1 = {mt: pers.tile([128, 1], F32, tag=f"a1{mt}", name=f"a1{mt}") for mt in range(MT)}
        c1 = {mt: pers.tile([128, 1], F32, tag=f"c1v{mt}", name=f"c1v{mt}") for mt in range(MT)}

        def ar1_pre(mt):
            local_sums(bnb1, mt, ar1[mt][:, 0:1], ar1[mt][:, 1:2])
            nc.sync.dma_start(cc1_in[mt][:], ar1[mt][:])
            nc.gpsimd.collective_compute(
                "AllReduce", ALU.add, replica_groups=groups,
                ins=[cc1_in[mt][:]], outs=[cc1_out[mt][:]],
            )

        def ar1_post(mt):
            nc.sync.dma_start(g1s[mt][:], cc1_out[mt][:])
            bn_affine(
                g1s[mt][:, 0:1], g1s[mt][:, 1:2],
                chv[mt][:, 1:2], chv[mt][:, 2:3], a1[mt][:], c1[mt][:],
            )

        def silu_both():
            # mt0/mt1 interleaved per tile: conv2 tile j needs both halves of
            # tile j only, so it unblocks right after AllReduce#1(mt1) lands
            for b in range(NB):
                for rg in range(RG):
                    for mt in range(MT):
                        s = t1[b, mt][:, rg * TLEN : (rg + 1) * TLEN]
                        nc.scalar.activation(
                            s, s, AF.Silu, bias=c1[mt][:], scale=a1[mt][:]
                        )

        # shortcut conv (1x1): PE + DVE bnstats + Pool bf16 staging only —
        # nothing here may depend on the AllReduce.
        def sc_mt(mt):
            for b in range(NB):
                for rg in range(RG):
                    ps = psB.tile([128, TLEN], F32, tag="sc", bufs=3, name="sc")
                    for kt in range(MT):
                        nc.tensor.matmul(
                            ps[:],
                            wsc[kt, mt],
                            win(xm[b, kt], rg),
                            start=(kt == 0),
                            stop=(kt == MT - 1),
                        )
                    _bn_stats_raw(nc, bnbs[mt][:, b * RG + rg, :], ps[:])

        conv1_mt(0)
        ar1_pre(0)           # AllReduce(mt0) flies while conv1(mt1) runs
        conv1_mt(1)
        ar1_pre(1)
        sc_mt(0)             # PE fill for the AllReduce window
        sc_mt(1)
        nc.scalar.activation(dummy[:], dummy[:], AF.Silu)  # warm table pre-AR
        ar1_post(0)
        ar1_post(1)
        silu_both()

        # conv2 stats pass: PE + DVE bnstats + Pool bf16 staging
        def conv2_mt(mt):
            for b in range(NB):
                for rg in range(RG):
                    ps = psB.tile([128, TLEN], F32, tag="z2", name="z2", bufs=3)
                    for kt in range(MT):
                        nc.tensor.matmul(
                            ps[:],
                            w2[kt, mt],
                            t1[b, kt][:, rg * TLEN : (rg + 1) * TLEN],
                            start=(kt == 0),
                            stop=(kt == MT - 1),
                        )
                    _bn_stats_raw(nc, bnb2[mt][:, b * RG + rg, :], ps[:])

        ar2 = {mt: pers.tile([128, 4], F32, tag=f"ar2{mt}", name=f"ar2{mt}") for mt in range(MT)}
        g2s = {mt: pers.tile([128, 4], F32, tag=f"g2s{mt}", name=f"g2s{mt}") for mt in range(MT)}
        a2 = {mt: pers.tile([128, 1], F32, tag=f"a2{mt}", name=f"a2{mt}") for mt in range(MT)}
        c2 = {mt: pers.tile([128, 1], F32, tag=f"c2{mt}", name=f"c2{mt}") for mt in range(MT)}
        asc = {mt: pers.tile([128, 1], F32, tag=f"as{mt}", name=f"as{mt}") for mt in range(MT)}
        ccb = {mt: pers.tile([128, 1], F32, tag=f"ccb{mt}", name=f"ccb{mt}") for mt in range(MT)}

        def ar2_pre(mt):
            local_sums(bnb2, mt, ar2[mt][:, 0:1], ar2[mt][:, 2:3])
            local_sums(bnbs, mt, ar2[mt][:, 1:2], ar2[mt][:, 3:4])
            nc.sync.dma_start(cc2_in[mt][:], ar2[mt][:])
            nc.gpsimd.collective_compute(
                "AllReduce", ALU.add, replica_groups=groups,
                ins=[cc2_in[mt][:]], outs=[cc2_out[mt][:]],
            )

        def ar2_dma(mt):
            nc.gpsimd.dma_start(g2s[mt][:], cc2_out[mt][:])

        def ar2_calc(mt):
            gsrc = g2s[mt][:]
            if mt == 1:
                # in-order DVE stream: without a data dep the tile scheduler
                # interleaves this chain ahead of calc(0)'s tail, stalling it
                # (and the PE behind it) until AllReduce#2(mt1) lands.
                g2b = stage.tile([128, 4], F32, tag="g2b", name="g2b", bufs=1)
                nc.vector.scalar_tensor_tensor(
                    g2b[:], wsc[MT - 1, 0][:, 0:4], 0.0, g2s[mt][:],
                    op0=ALU.mult, op1=ALU.add,
                )
                gsrc = g2b[:]
            ap = stage.tile([128, 2], F32, tag="apk", name="apk", bufs=2)
            cp = stage.tile([128, 2], F32, tag="cpk", name="cpk", bufs=2)
            bn_affine(
                gsrc[:, 0:2], gsrc[:, 2:4],
                chv[mt][:, 8:10], chv[mt][:, 10:12], ap[:], cp[:], wdt=2,
            )
            nc.vector.tensor_copy(a2[mt][:], ap[:, 0:1])
            nc.vector.tensor_tensor(
                ccb[mt][:], cp[:, 0:1], cp[:, 1:2], op=ALU.add
            )
            # r = asc/a2; wsc *= r broadcast along output channels, so the
            # final pass is one PSUM accumulation evacuated by a single
            # silu(a2*ps + ccb) ACT op. Broadcast: PE transpose (r -> row)
            # then a rank-1 ones x rT matmul; wsc scaling reads PSUM direct.
            r = stage.tile([128, 1], F32, tag="rr", name="rr")
            nc.vector.reciprocal(r[:], ap[:, 0:1])
            nc.vector.tensor_tensor(r[:], r[:], ap[:, 1:2], op=ALU.mult)
            psT = psC.tile([1, 128], F32, tag="pst", name="pst", bufs=1)
            nc.tensor.transpose(psT[:], r[:], idn[:])
            rT = stage.tile([1, 128], F32, tag="rT", name="rT", bufs=2)
            nc.vector.tensor_copy(rT[:], psT[:])
            psR = psC.tile([128, 128], F32, tag="rps", name="rps", bufs=1)
            nc.tensor.matmul(psR[:], ones_row[:], rT[:], start=True, stop=True)
            for kt in range(MT):
                nc.vector.tensor_tensor(wsc[kt, mt], wsc[kt, mt], psR[:], op=ALU.mult)

        conv2_mt(0)
        ar2_pre(0)           # flies while conv2(1) runs
        conv2_mt(1)
        ar2_pre(1)
        esB.close()

        esC = ExitStack()
        psC = esC.enter_context(tc.tile_pool(name="psC", bufs=4, space="PSUM"))

        # =====================================================================
        # Phase C: final fused pass  y = silu(a2*z2 + as*sc + (c2+cs))
        # pure DVE/ACT from the bf16 caches; mt0 runs while AllReduce#2(mt1)
        # is still in flight.
        # =====================================================================
        PF = 6

        def final_prefill(mt):
            tiles = []
            for b in range(NB):
                for rg in range(RG):
                    tiles.append((b, rg, psC.tile([128, TLEN], F32, tag="fz", name="fz", bufs=PF)))
            for b, rg, ps in tiles[:PF]:
                for kt in range(MT):
                    nc.tensor.matmul(
                        ps[:], w2[kt, mt],
                        t1[b, kt][:, rg * TLEN : (rg + 1) * TLEN],
                        start=(kt == 0), stop=False,
                    )
            return tiles

        def final_rest(mt, tiles):
            for i, (b, rg, ps) in enumerate(tiles):
                if i >= PF:
                    for kt in range(MT):
                        nc.tensor.matmul(
                            ps[:], w2[kt, mt],
                            t1[b, kt][:, rg * TLEN : (rg + 1) * TLEN],
                            start=(kt == 0), stop=False,
                        )
                for kt in range(MT):
                    nc.tensor.matmul(
                        ps[:], wsc[kt, mt], win(xm[b, kt], rg),
                        start=False, stop=(kt == MT - 1),
                    )
                stg = stage.tile([128, TLEN], F32, tag="stg", name="stg", bufs=6)
                nc.scalar.activation(
                    stg[:], ps[:], AF.Silu, bias=ccb[mt][:], scale=a2[mt][:]
                )
                yq3 = [nc.sync, nc.scalar, nc.gpsimd]
                yeng = yq3[(i + (0 if mt == 0 else 1)) % 3]
                yeng.dma_start(
                    y_d[b, mt * 128 : mt * 128 + 128, rg * RGR : (rg + 1) * RGR, :],
                    stg[:],
                )

        ar2_dma(0)
        ar2_dma(1)
        t0 = final_prefill(0)
        ar2_calc(0)
        final_rest(0, t0)
        t1f = final_prefill(1)
        ar2_calc(1)
        final_rest(1, t1f)
        esC.close()

    _split_sync_waits(nc)
    return nc


_NC = None


def _round_f32r(a):
    """Round-to-nearest-even to 11 mantissa bits (the PE's f32r format,
    verified bit-exact against the hardware cast DMA)."""
    x = np.ascontiguousarray(a, dtype=np.float32).view(np.uint32).astype(np.uint64)
    half = np.uint64(1 << 11)
    lowmask = np.uint64((1 << 12) - 1)
    lsb = np.uint64(1 << 12)
    exact_half = (x & lowmask) == half
    r = np.where(exact_half & ((x & lsb) == np.uint64(0)), x, x + half)
    r = r & np.uint64(0xFFFFF000)
    return r.astype(np.uint32).view(np.float32).reshape(a.shape)


def _prep_inputs(inputs):
    bf = ml_dtypes.bfloat16
    w_conv1 = inputs["w_conv1"]  # (C, C, 3, 3) OIHW
    w1t = np.ascontiguousarray(
        w_conv1.transpose(2, 3, 1, 0).reshape(9, C, C)
    ).astype(bf)  # [tap(kh*3+kw), cin, cout]
    w2t = np.ascontiguousarray(inputs["w_conv2"][:, :, 0, 0].T).astype(bf)
    wsct = np.ascontiguousarray(inputs["w_sc"][:, :, 0, 0].T).astype(bf)
    wdce_t = np.ascontiguousarray(inputs["w_dce"].T).astype(np.float32)
    wst = np.ascontiguousarray(inputs["w_shrink"].T).astype(np.float32)
    wet = np.ascontiguousarray(inputs["w_expand"].T).astype(np.float32)

    wch = inputs["w_ch"][:, 0]  # (C, 3, 3)
    # gath order: [S, rowE, row0, colE, col0, x(E,E), x(E,0), x(0,E), x(0,0)]
    chc = np.stack(
        [
            wch.sum((1, 2)),
            -wch[:, 0, :].sum(1),
            -wch[:, 2, :].sum(1),
            -wch[:, :, 0].sum(1),
            -wch[:, :, 2].sum(1),
            wch[:, 0, 0],
            wch[:, 0, 2],
            wch[:, 2, 0],
            wch[:, 2, 2],
        ],
        axis=1,
    ).astype(np.float32) / float(H * W)

    chv = np.stack(
        [
            inputs["b_dce"], inputs["g_bn1"], inputs["be_bn1"],
            inputs["g_bn2"], inputs["be_bn2"], inputs["g_bns"],
            inputs["be_bns"], inputs["b_expand"],
            inputs["g_bn2"], inputs["g_bns"], inputs["be_bn2"], inputs["be_bns"],
        ],
        axis=1,
    ).astype(np.float32)

    shared = {
        "w1t": w1t, "w2t": w2t, "wsct": wsct, "wdce_t": wdce_t,
        "wst": wst, "wet": wet, "chc": np.ascontiguousarray(chc),
        "chv": np.ascontiguousarray(chv),
        "bsh": inputs["b_shrink"].astype(np.float32),
    }
    in_maps = []
    for c in range(NCORES):
        m = dict(shared)
        xc = inputs["x"][c * NB : (c + 1) * NB]
        xp = np.zeros((NB, C, PLEN), np.float32)
        xp[:, :, : (H + 2) * PW].reshape(NB, C, H + 2, PW)[
            :, :, 1 : H + 1, 1 : W + 1
        ] = xc
        m["xp"] = xp.astype(bf)
        m["dce"] = np.ascontiguousarray(
            inputs["dce_output"][c * NB : (c + 1) * NB]
        ).astype(np.float32)
        in_maps.append(m)
    return in_maps


def kernel(**inputs):
    global _NC
    if _NC is None:
        _NC = _build()
    in_maps = _prep_inputs(inputs)
    res = run_bass_kernel_spmd(_NC, in_maps, list(range(NCORES)))
    return np.concatenate([res.results[c]["y"] for c in range(NCORES)], axis=0)


if __name__ == "__main__":
    nc = _build()
    print("build ok")


# revision 18
# speedup vs baseline: 1.1945x; 1.1945x over previous
"""DCE-modulated ResBlock (dense_cnn) on 8 Trainium2 NeuronCores.

Data-parallel over batch (16 images -> 2 per core), weights replicated.
BatchNorm batch statistics are made exact via two cross-core AllReduces
(sync-BN): one for bn1, one for bn2+shortcut-bn.

v3 vs baseline:
- shortcut conv runs entirely BEFORE any AllReduce-dependent vector op:
  in-order engine streams mean ar1_post's bn_affine head-of-line blocked
  the DVE bnstats the shortcut needed, so the PE idled through the whole
  AllReduce wait in the old ordering.
- conv2/shortcut outputs are cached to SBUF in bf16 by the ACT engine
  during the stats pass; the final pass is then pure DVE/ACT affine+silu
  with no recomputation matmuls. t1/w2 are bf16 (same PE rate, half SBUF
  and DMA); x/xm/w1/wsc stay f32r because DVE/Pool *elementwise* on bf16
  measured 2.5-20x slower than f32.
- bn_affine's rsqrt runs on DVE (pow -0.5): keeps Sqrt off the ACT table
  cache so Silu/Sigmoid never reload mid-critical-path; a dummy Silu op
  pre-warms the table during the AllReduce wait.
- final pass is an ACT-led 2-op pipeline with the DVE add in between.
"""

from contextlib import ExitStack

import numpy as np
import ml_dtypes

import concourse.bass as bass
import concourse.mybir as mybir
from concourse import tile
from concourse.bass_utils import run_bass_kernel_spmd

F32 = mybir.dt.float32
F32R = mybir.dt.float32r
BF16 = mybir.dt.bfloat16
AF = mybir.ActivationFunctionType
ALU = mybir.AluOpType

B, C, H, W = 16, 256, 64, 64
LDCE, CDCE = 100, 128
NCORES = 8
NB = B // NCORES          # images per core
MT = C // 128             # channel tiles (2)
PW = W + 2                # padded row width 66
PLEN = (H + 2) * PW + 2   # padded buffer + 2 guard cols (4358)
RG = 8                    # row groups per image
RGR = H // RG             # rows per group (8)
TLEN = RGR * W            # columns per psum tile (512)
TSP = RGR * PW            # padded columns spanned per group (528)
NLOC = NB * H * W         # local reduction count per channel (8192)
NGLB = B * H * W          # global reduction count (65536)
EPS = 1e-5
XSPLIT = 67 + 32 * PW     # 2179: row 0-31 sums need only the first chunk


def _split_sync_waits(nc, max_waits=1):
    """This container's walrus build accepts only one sync-wait command per
    instruction; hoist excess waits onto same-engine NoOps placed before."""
    for f in nc.m.functions:
        for bb in f.blocks:
            insts = bb.instructions
            if not any(
                i.sync_info is not None and len(i.sync_info.on_wait) > max_waits
                for i in insts
            ):
                continue
            newlist = []
            for inst in insts:
                si = inst.sync_info
                if si is not None and len(si.on_wait) > max_waits:
                    waits = list(si.on_wait)
                    extra, keep = waits[:-max_waits], waits[-max_waits:]
                    for j in range(0, len(extra), max_waits):
                        nop = mybir.InstNoOp(name=f"{inst.name}-sw{j}", ins=[], outs=[])
                        nop.engine = inst.engine
                        nop.sync_info = mybir.SyncInfo(
                            on_wait=extra[j : j + max_waits], on_update=[]
                        )
                        newlist.append(nop)
                    inst.sync_info = mybir.SyncInfo(
                        on_wait=keep, on_update=list(si.on_update)
                    )
                newlist.append(inst)
            bb.instructions = newlist


def _bn_stats_raw(nc, out_ap, in_ap):
    """One HW BNStats chunk (count/mean/count*var for even+odd lanes) over
    the full (possibly strided) input AP; bass's shape assert only allows
    flat inputs, walrus only allows 6 outputs, so emit the IR directly."""
    eng = nc.vector
    eng.add_instruction(
        mybir.InstBNStats(
            name=nc.get_next_instruction_name(),
            ins=[eng.lower_ap(in_ap)],
            outs=[eng.lower_ap(out_ap)],
        )
    )


def _build():
    nc = bass.Bass(
        "TRN2",
        target_bir_lowering=False,
        debug=False,
        num_devices=NCORES,
        use_seq_codegen=True,
        num_swdge_queues=4,
    )

    # ---- kernel I/O (per-core shapes) ----
    xp_d = nc.dram_tensor("xp", [NB, C, PLEN], BF16, kind="ExternalInput")
    dce_d = nc.dram_tensor("dce", [NB, LDCE, CDCE], F32, kind="ExternalInput")
    w1t_d = nc.dram_tensor("w1t", [9, C, C], BF16, kind="ExternalInput")
    w2t_d = nc.dram_tensor("w2t", [C, C], BF16, kind="ExternalInput")
    wsct_d = nc.dram_tensor("wsct", [C, C], BF16, kind="ExternalInput")
    wdce_d = nc.dram_tensor("wdce_t", [CDCE, C], F32, kind="ExternalInput")
    wst_d = nc.dram_tensor("wst", [C, C // 2], F32, kind="ExternalInput")
    wet_d = nc.dram_tensor("wet", [C // 2, C], F32, kind="ExternalInput")
    chc_d = nc.dram_tensor("chc", [C, 9], F32, kind="ExternalInput")
    # per-channel vectors: [b_dce, g1, be1, g2, be2, gs, bes, b_expand]
    chv_d = nc.dram_tensor("chv", [C, 12], F32, kind="ExternalInput")
    bsh_d = nc.dram_tensor("bsh", [C // 2], F32, kind="ExternalInput")
    y_d = nc.dram_tensor("y", [NB, C, H, W], F32, kind="ExternalOutput")

    # collective bounce buffers (one pair per AllReduce so they pipeline)
    cc1_in = {mt: nc.dram_tensor(f"cc1_in{mt}", [128, 2], F32) for mt in range(MT)}
    cc1_out = {
        mt: nc.dram_tensor(f"cc1_out{mt}", [128, 2], F32, addr_space="Shared")
        for mt in range(MT)
    }
    cc2_in = {mt: nc.dram_tensor(f"cc2_in{mt}", [128, 4], F32) for mt in range(MT)}
    cc2_out = {
        mt: nc.dram_tensor(f"cc2_out{mt}", [128, 4], F32, addr_space="Shared")
        for mt in range(MT)
    }
    groups = [list(range(NCORES))]

    with tile.TileContext(nc) as tc, ExitStack() as es:
        pers = es.enter_context(tc.tile_pool(name="pers", bufs=1))
        stage = es.enter_context(tc.tile_pool(name="stage", bufs=4))

        # ---- persistent SBUF buffers ----
        xm = {}   # padded x, later x*mod (f32r)
        t1 = {}   # conv1 out, later silu(bn1(.)) (bf16)
        for b in range(NB):
            for ct in range(MT):
                xm[b, ct] = pers.tile([128, PLEN], BF16, tag=f"xm{b}{ct}", name=f"xm{b}{ct}")
                t1[b, ct] = pers.tile([128, H * W], BF16, tag=f"t1{b}{ct}", name=f"t1{b}{ct}")

        # ---- ACT table preloads (hidden under the x DMA) ----
        dummy = pers.tile([128, 1], F32, tag="dummy", name="dummy")
        nc.vector.memset(dummy[:], 1.0)
        nc.scalar.activation(dummy[:], dummy[:], AF.Sigmoid)

        # ---- load x first (x gates the whole pipeline); 4 queues ----
        xq = {(0, 0): (nc.sync, nc.sync), (0, 1): (nc.scalar, nc.scalar),
              (1, 0): (nc.gpsimd, nc.gpsimd), (1, 1): (nc.sync, nc.scalar)}
        for b in range(NB):
            for ct in range(MT):
                e1, e2 = xq[b, ct]
                e1.dma_start(
                    xm[b, ct][:, 0:XSPLIT],
                    xp_d[b, ct * 128 : ct * 128 + 128, 0:XSPLIT],
                )
                e2.dma_start(
                    xm[b, ct][:, XSPLIT:PLEN],
                    xp_d[b, ct * 128 : ct * 128 + 128, XSPLIT:PLEN],
                )

        w1 = {}
        for mt in range(MT):      # mt outer: conv1(mt0) weights land first
            for kt in range(MT):
                big = pers.tile([128, 9 * 128], BF16, tag=f"w1b{kt}{mt}", name=f"w1b{kt}{mt}")
                nc.gpsimd.dma_start(
                    big[:].rearrange("p (t o) -> p t o", t=9),
                    w1t_d[:, kt * 128 : kt * 128 + 128, mt * 128 : mt * 128 + 128]
                    .rearrange("t c o -> c t o"),
                )
                for tap in range(9):
                    w1[tap, kt, mt] = big[:, tap * 128 : (tap + 1) * 128]
        w2 = {}
        wsc = {}
        for kt in range(MT):
            bw = pers.tile([128, 2 * 128], BF16, tag=f"w2b{kt}", name=f"w2b{kt}")
            nc.sync.dma_start(
                bw[:].rearrange("p (m o) -> p m o", m=MT),
                w2t_d[kt * 128 : kt * 128 + 128, :].rearrange("c (m o) -> c m o", m=MT),
            )
            bs = pers.tile([128, 2 * 128], BF16, tag=f"wscb{kt}", name=f"wscb{kt}")
            nc.sync.dma_start(
                bs[:].rearrange("p (m o) -> p m o", m=MT),
                wsct_d[kt * 128 : kt * 128 + 128, :].rearrange("c (m o) -> c m o", m=MT),
            )
            for mt in range(MT):
                w2[kt, mt] = bw[:, mt * 128 : (mt + 1) * 128]
                wsc[kt, mt] = bs[:, mt * 128 : (mt + 1) * 128]
        wdce = {}
        wet = {}
        chv = {}
        chc = {}
        for mt in range(MT):
            wdce[mt] = pers.tile([128, 128], F32, tag=f"wdce{mt}", name=f"wdce{mt}")
            nc.sync.dma_start(wdce[mt][:], wdce_d[:, mt * 128 : mt * 128 + 128])
            wet[mt] = pers.tile([128, 128], F32, tag=f"wet{mt}", name=f"wet{mt}")
            nc.sync.dma_start(wet[mt][:], wet_d[:, mt * 128 : mt * 128 + 128])
            chv[mt] = pers.tile([128, 12], F32, tag=f"chv{mt}", name=f"chv{mt}")
            nc.sync.dma_start(chv[mt][:], chv_d[mt * 128 : mt * 128 + 128, :])
            chc[mt] = pers.tile([128, 9], F32, tag=f"chc{mt}", name=f"chc{mt}")
            nc.sync.dma_start(chc[mt][:], chc_d[mt * 128 : mt * 128 + 128, :])
        wst = {}
        for kt in range(MT):
            wst[kt] = pers.tile([128, 128], F32, tag=f"wst{kt}", name=f"wst{kt}")
            nc.sync.dma_start(wst[kt][:], wst_d[kt * 128 : kt * 128 + 128, :])
        bsh = pers.tile([128, 1], F32, tag="bsh", name="bsh")
        nc.sync.dma_start(bsh[:], bsh_d[:].rearrange("(p a) -> p a", a=1))

        # =====================================================================
        # Phase A: modulation gate
        # =====================================================================
        esA = ExitStack()
        psA = esA.enter_context(tc.tile_pool(name="psA", bufs=2, space="PSUM"))

        ones_f = pers.tile([128, 1], F32, tag="ones_f", name="ones_f")
        nc.vector.memset(ones_f[:], 1.0)
        ones_row = pers.tile([1, 128], F32, tag="ones_row", name="ones_row")
        nc.vector.memset(ones_row[:], 1.0)
        idn = pers.tile([128, 128], F32, tag="idn", name="idn")
        iot = pers.tile([128, 128], mybir.dt.int32, tag="iot", name="iot")
        nc.gpsimd.iota(iot[:], [[1, 128]], channel_multiplier=-1)
        nc.vector.tensor_scalar(idn[:], iot[:], 0, None, op0=ALU.is_equal)

        pooled = pers.tile([128, NB], F32, tag="pooled", name="pooled")  # dce mean, CDCE x img
        for b in range(NB):
            dce_sb = stage.tile([LDCE, CDCE], F32, tag="dce_sb", name="dce_sb")
            nc.sync.dma_start(dce_sb[:], dce_d[b, :, :])
            ps = psA.tile([128, 1], F32, tag="tiny", name="tiny")
            nc.tensor.matmul(ps[:], dce_sb[:], ones_f[0:LDCE, :], start=True, stop=True)
            nc.scalar.mul(pooled[:, b : b + 1], ps[:], 1.0 / LDCE)

        # spatial_proj via border-sum identity; gath cols:
        # [S, rowE, row0, colE, col0, x(E,E), x(E,0), x(0,E), x(0,0)]
        # row sums include the zero padding cols, so one [64,66] reduce works;
        # rows 0-31 need only the first x chunk, rows 32-63 go on Pool.
        sp = {}
        for ct in range(MT):
            sp[ct] = pers.tile([128, NB], F32, tag=f"sp{ct}", name=f"sp{ct}")
        for b in range(NB):
            for ct in range(MT):
                buf = xm[b, ct]
                gath = stage.tile([128, 9], F32, tag="gath", name="gath")
                rows = stage.tile([128, H], F32, tag="rows", name="rows")
                halfA = buf[:, 66 : 66 + 32 * PW].rearrange("p (r c) -> p r c", r=32)
                halfB = buf[:, 66 + 32 * PW : 66 + 64 * PW].rearrange(
                    "p (r c) -> p r c", r=32
                )
                nc.vector.reduce_sum(rows[:, 0:32], halfA, axis=mybir.AxisListType.X)
                nc.vector.reduce_sum(rows[:, 32:64], halfB, axis=mybir.AxisListType.X)
                nc.vector.reduce_sum(gath[:, 0:1], rows[:], axis=mybir.AxisListType.X)
                nc.vector.tensor_copy(gath[:, 1:2], rows[:, H - 1 : H])
                nc.vector.tensor_copy(gath[:, 2:3], rows[:, 0:1])
                colE = buf[:, 67 + W - 1 : 67 + W - 1 + H * PW].rearrange(
                    "p (r c) -> p r c", r=H
                )[:, :, 0:1]
                col0 = buf[:, 67 : 67 + H * PW].rearrange(
                    "p (r c) -> p r c", r=H
                )[:, :, 0:1]
                nc.vector.reduce_sum(gath[:, 3:4], colE, axis=mybir.AxisListType.XY)
                nc.vector.reduce_sum(gath[:, 4:5], col0, axis=mybir.AxisListType.XY)
                be = 67 + (H - 1) * PW
                nc.vector.tensor_copy(gath[:, 5:6], buf[:, be + W - 1 : be + W])
                nc.vector.tensor_copy(gath[:, 6:7], buf[:, be : be + 1])
                nc.vector.tensor_copy(gath[:, 7:8], buf[:, 67 + W - 1 : 67 + W])
                nc.vector.tensor_copy(gath[:, 8:9], buf[:, 67 : 68])
                gm = stage.tile([128, 9], F32, tag="gm", name="gm")
                nc.vector.tensor_tensor(gm[:], gath[:], chc[ct][:], op=ALU.mult)
                nc.vector.reduce_sum(
                    sp[ct][:, b : b + 1], gm[:], axis=mybir.AxisListType.X
                )

        # m = (dce_pooled @ w_dce.T + b_dce) * spatial_proj   -> [128, NB]/ct
        m_r = {}
        for mt in range(MT):
            ps = psA.tile([128, NB], F32, tag="tiny", name="tiny")
            nc.tensor.matmul(ps[:], wdce[mt][:], pooled[:], start=True, stop=True)
            dcep = stage.tile([128, NB], F32, tag="dcep", name="dcep")
            nc.scalar.add(dcep[:], ps[:], chv[mt][:, 0:1])
            m_r[mt] = pers.tile([128, NB], F32, tag=f"m{mt}", name=f"m{mt}")
            nc.vector.tensor_tensor(m_r[mt][:], dcep[:], sp[mt][:], op=ALU.mult)

        # h = relu(m @ w_shrink.T + b_shrink)  -> [128, NB]
        ps_h = psA.tile([128, NB], F32, tag="tiny", name="tiny")
        for kt in range(MT):
            nc.tensor.matmul(
                ps_h[:], wst[kt][:], m_r[kt][:], start=(kt == 0), stop=(kt == MT - 1)
            )
        h_r = pers.tile([128, NB], F32, tag="h_r", name="h_r")
        nc.scalar.activation(h_r[:], ps_h[:], AF.Relu, bias=bsh[:])

        # mod = sigmoid(h @ w_expand.T + b_expand)  -> f32 [128, NB] per ct
        mod = {}
        for mt in range(MT):
            ps = psA.tile([128, NB], F32, tag="tiny", name="tiny")
            nc.tensor.matmul(ps[:], wet[mt][:], h_r[:], start=True, stop=True)
            mod[mt] = pers.tile([128, NB], F32, tag=f"mod{mt}", name=f"mod{mt}")
            nc.scalar.activation(mod[mt][:], ps[:], AF.Sigmoid, bias=chv[mt][:, 7:8])

        # xm = x * mod (in place, chunked so conv1's first windows unblock
        # early; chunks split over DVE/Pool/ACT)
        XCH = [0, 1190, 2180, 3270, PLEN]
        for i in range(len(XCH) - 1):
            for b in range(NB):
                for ct in range(MT):
                    s, e = XCH[i], XCH[i + 1]
                    nc.scalar.activation(
                        xm[b, ct][:, s:e], xm[b, ct][:, s:e],
                        AF.Copy, scale=mod[ct][:, b : b + 1],
                    )
        esA.close()

        # =====================================================================
        # Phase B: conv1 (+bn1 stats); shortcut conv fully before any
        # AllReduce-dependent op so the PE fills the AllReduce wait.
        # =====================================================================
        bnb1 = {mt: pers.tile([128, NB * RG, 6], F32, tag=f"bnb1{mt}", name=f"bnb1{mt}") for mt in range(MT)}
        bnbs = {mt: pers.tile([128, NB * RG, 6], F32, tag=f"bnbs{mt}", name=f"bnbs{mt}") for mt in range(MT)}
        bnb2 = {mt: pers.tile([128, NB * RG, 6], F32, tag=f"bnb2{mt}", name=f"bnb2{mt}") for mt in range(MT)}

        taps = [((kh - 1) * PW + (kw - 1), 3 * kh + kw) for kh in range(3) for kw in range(3)]

        esB = ExitStack()
        psB = esB.enter_context(tc.tile_pool(name="psB", bufs=2, space="PSUM"))

        def win(buf, rg, off=0):
            s = 67 + rg * TSP + off
            return buf[:, s : s + RGR * PW].rearrange("p (r c) -> p r c", r=RGR)[
                :, :, 0:W
            ]

        def conv1_mt(mt):
            for b in range(NB):
                for rg in range(RG):
                    ps = psB.tile([128, TLEN], F32, tag="c1", name="c1", bufs=2)
                    first = True
                    for kt in range(MT):
                        for off, tap in taps:
                            nc.tensor.matmul(
                                ps[:],
                                w1[tap, kt, mt],
                                win(xm[b, kt], rg, off),
                                start=first,
                                stop=(kt == MT - 1 and tap == 8),
                            )
                            first = False
                    _bn_stats_raw(nc, bnb1[mt][:, b * RG + rg, :], ps[:])
                    nc.scalar.copy(
                        t1[b, mt][:, rg * TLEN : (rg + 1) * TLEN], ps[:]
                    )

        # local chunk stats -> (sum, sum_x2) packed for the allreduce
        def local_sums(bnb, mt, dst_sum, dst_ex2):
            mv = stage.tile([128, 2], F32, tag="mv", name="mv")
            nc.vector.bn_aggr(
                mv[:],
                bnb[mt][:]
                .rearrange("p a s -> p (a s)")
                .rearrange("p (a b) -> p a b", b=3),
            )
            nc.vector.tensor_scalar_mul(dst_sum, mv[:, 0:1], float(NLOC))
            t = stage.tile([128, 1], F32, tag="tloc", name="tloc")
            nc.vector.tensor_tensor(t[:], mv[:, 0:1], mv[:, 0:1], op=ALU.mult)
            nc.vector.tensor_tensor(t[:], t[:], mv[:, 1:2], op=ALU.add)
            nc.vector.tensor_scalar_mul(dst_ex2, t[:], float(NLOC))

        # global bn affine: a = g*rsqrt(var+eps), c = be - mean*a
        def bn_affine(sum_ap, ex2_ap, g_ap, be_ap, a_dst, c_dst, wdt=1):
            mean = stage.tile([128, wdt], F32, tag=f"bnm{wdt}", name="bnm")
            nc.vector.tensor_scalar_mul(mean[:], sum_ap, 1.0 / NGLB)
            var = stage.tile([128, wdt], F32, tag=f"bnv{wdt}", name="bnv")
            nc.vector.tensor_scalar_mul(var[:], ex2_ap, 1.0 / NGLB)
            t = stage.tile([128, wdt], F32, tag=f"bnt{wdt}", name="bnt")
            nc.vector.tensor_tensor(t[:], mean[:], mean[:], op=ALU.mult)
            nc.vector.tensor_tensor(var[:], var[:], t[:], op=ALU.subtract)
            nc.vector.tensor_scalar_add(var[:], var[:], EPS)
            # rsqrt on DVE (bit-trick + 2 Newton steps) so the ACT table
            # cache never has to swap Silu out for Sqrt mid-critical-path
            yq = stage.tile([128, wdt], F32, tag=f"yq{wdt}", name="yq")
            nc.vector.tensor_scalar(
                yq[:].bitcast(mybir.dt.int32), var[:].bitcast(mybir.dt.int32),
                1, None, op0=ALU.logical_shift_right,
            )
            nc.vector.tensor_scalar(
                yq[:].bitcast(mybir.dt.int32), yq[:].bitcast(mybir.dt.int32),
                -1, 0x5F3759DF, op0=ALU.mult, op1=ALU.add,
            )
            h = stage.tile([128, wdt], F32, tag=f"hq{wdt}", name="hq")
            nc.vector.tensor_scalar_mul(h[:], var[:], 0.5)
            t2 = stage.tile([128, wdt], F32, tag=f"t2q{wdt}", name="t2q")
            for _ in range(2):
                nc.vector.tensor_tensor(t2[:], yq[:], yq[:], op=ALU.mult)
                nc.vector.tensor_tensor(t2[:], t2[:], h[:], op=ALU.mult)
                nc.vector.tensor_scalar(
                    t2[:], t2[:], -1.0, 1.5, op0=ALU.mult, op1=ALU.add
                )
                nc.vector.tensor_tensor(yq[:], yq[:], t2[:], op=ALU.mult)
            nc.vector.tensor_tensor(a_dst, yq[:], g_ap, op=ALU.mult)
            nc.vector.tensor_tensor(t[:], mean[:], a_dst, op=ALU.mult)
            nc.vector.tensor_tensor(c_dst, be_ap, t[:], op=ALU.subtract)

        ar1 = {mt: pers.tile([128, 2], F32, tag=f"ar1{mt}", name=f"ar1{mt}") for mt in range(MT)}
        g1s = {mt: pers.tile([128, 2], F32, tag=f"g1s{mt}", name=f"g1s{mt}") for mt in range(MT)}
        a1 = {mt: pers.tile([128, 1], F32, tag=f"a1{mt}", name=f"a1{mt}") for mt in range(MT)}
        c1 = {mt: pers.tile([128, 1], F32, tag=f"c1v{mt}", name=f"c1v{mt}") for mt in range(MT)}

        def ar1_pre(mt):
            local_sums(bnb1, mt, ar1[mt][:, 0:1], ar1[mt][:, 1:2])
            nc.sync.dma_start(cc1_in[mt][:], ar1[mt][:])
            nc.gpsimd.collective_compute(
                "AllReduce", ALU.add, replica_groups=groups,
                ins=[cc1_in[mt][:]], outs=[cc1_out[mt][:]],
            )

        def ar1_post(mt):
            nc.sync.dma_start(g1s[mt][:], cc1_out[mt][:])
            bn_affine(
                g1s[mt][:, 0:1], g1s[mt][:, 1:2],
                chv[mt][:, 1:2], chv[mt][:, 2:3], a1[mt][:], c1[mt][:],
            )

        def silu_both():
            # mt0/mt1 interleaved per tile: conv2 tile j needs both halves of
            # tile j only, so it unblocks right after AllReduce#1(mt1) lands
            for b in range(NB):
                for rg in range(RG):
                    for mt in range(MT):
                        s = t1[b, mt][:, rg * TLEN : (rg + 1) * TLEN]
                        nc.scalar.activation(
                            s, s, AF.Silu, bias=c1[mt][:], scale=a1[mt][:]
                        )

        # shortcut conv (1x1): PE + DVE bnstats + Pool bf16 staging only —
        # nothing here may depend on the AllReduce.
        def sc_mt(mt):
            for b in range(NB):
                for rg in range(RG):
                    ps = psB.tile([128, TLEN], F32, tag="sc", bufs=3, name="sc")
                    for kt in range(MT):
                        nc.tensor.matmul(
                            ps[:],
                            wsc[kt, mt],
                            win(xm[b, kt], rg),
                            start=(kt == 0),
                            stop=(kt == MT - 1),
                        )
                    _bn_stats_raw(nc, bnbs[mt][:, b * RG + rg, :], ps[:])

        conv1_mt(0)
        ar1_pre(0)           # AllReduce(mt0) flies while conv1(mt1) runs
        conv1_mt(1)
        ar1_pre(1)
        sc_mt(0)             # PE fill for the AllReduce window
        sc_mt(1)
        nc.scalar.activation(dummy[:], dummy[:], AF.Silu)  # warm table pre-AR
        ar1_post(0)
        ar1_post(1)
        silu_both()

        # conv2 stats pass: PE + DVE bnstats + Pool bf16 staging
        def conv2_mt(mt):
            for b in range(NB):
                for rg in range(RG):
                    ps = psB.tile([128, TLEN], F32, tag="z2", name="z2", bufs=3)
                    for kt in range(MT):
                        nc.tensor.matmul(
                            ps[:],
                            w2[kt, mt],
                            t1[b, kt][:, rg * TLEN : (rg + 1) * TLEN],
                            start=(kt == 0),
                            stop=(kt == MT - 1),
                        )
                    _bn_stats_raw(nc, bnb2[mt][:, b * RG + rg, :], ps[:])

        ar2 = {mt: pers.tile([128, 4], F32, tag=f"ar2{mt}", name=f"ar2{mt}") for mt in range(MT)}
        g2s = {mt: pers.tile([128, 4], F32, tag=f"g2s{mt}", name=f"g2s{mt}") for mt in range(MT)}
        a2 = {mt: pers.tile([128, 1], F32, tag=f"a2{mt}", name=f"a2{mt}") for mt in range(MT)}
        c2 = {mt: pers.tile([128, 1], F32, tag=f"c2{mt}", name=f"c2{mt}") for mt in range(MT)}
        asc = {mt: pers.tile([128, 1], F32, tag=f"as{mt}", name=f"as{mt}") for mt in range(MT)}
        ccb = {mt: pers.tile([128, 1], F32, tag=f"ccb{mt}", name=f"ccb{mt}") for mt in range(MT)}

        def ar2_pre(mt):
            local_sums(bnb2, mt, ar2[mt][:, 0:1], ar2[mt][:, 2:3])
            local_sums(bnbs, mt, ar2[mt][:, 1:2], ar2[mt][:, 3:4])
            nc.sync.dma_start(cc2_in[mt][:], ar2[mt][:])
            nc.gpsimd.collective_compute(
                "AllReduce", ALU.add, replica_groups=groups,
                ins=[cc2_in[mt][:]], outs=[cc2_out[mt][:]],
            )

        def ar2_dma(mt):
            nc.gpsimd.dma_start(g2s[mt][:], cc2_out[mt][:])

        def ar2_calc(mt):
            gsrc = g2s[mt][:]
            if mt == 1:
                # in-order DVE stream: without a data dep the tile scheduler
                # interleaves this chain ahead of calc(0)'s tail, stalling it
                # (and the PE behind it) until AllReduce#2(mt1) lands.
                g2b = stage.tile([128, 4], F32, tag="g2b", name="g2b", bufs=1)
                nc.vector.scalar_tensor_tensor(
                    g2b[:], wsc[MT - 1, 0][:, 0:4], 0.0, g2s[mt][:],
                    op0=ALU.mult, op1=ALU.add,
                )
                gsrc = g2b[:]
            ap = stage.tile([128, 2], F32, tag="apk", name="apk", bufs=2)
            cp = stage.tile([128, 2], F32, tag="cpk", name="cpk", bufs=2)
            bn_affine(
                gsrc[:, 0:2], gsrc[:, 2:4],
                chv[mt][:, 8:10], chv[mt][:, 10:12], ap[:], cp[:], wdt=2,
            )
            nc.vector.tensor_copy(a2[mt][:], ap[:, 0:1])
            nc.vector.tensor_tensor(
                ccb[mt][:], cp[:, 0:1], cp[:, 1:2], op=ALU.add
            )
            # r = asc/a2; wsc *= r broadcast along output channels, so the
            # final pass is one PSUM accumulation evacuated by a single
            # silu(a2*ps + ccb) ACT op. Broadcast: PE transpose (r -> row)
            # then a rank-1 ones x rT matmul; wsc scaling reads PSUM direct.
            r = stage.tile([128, 1], F32, tag="rr", name="rr")
            nc.vector.reciprocal(r[:], ap[:, 0:1])
            nc.vector.tensor_tensor(r[:], r[:], ap[:, 1:2], op=ALU.mult)
            psT = psC.tile([1, 128], F32, tag="pst", name="pst", bufs=1)
            nc.tensor.transpose(psT[:], r[:], idn[:])
            rT = stage.tile([1, 128], F32, tag="rT", name="rT", bufs=2)
            nc.vector.tensor_copy(rT[:], psT[:])
            psR = psC.tile([128, 128], F32, tag="rps", name="rps", bufs=1)
            nc.tensor.matmul(psR[:], ones_row[:], rT[:], start=True, stop=True)
            for kt in range(MT):
                nc.vector.tensor_tensor(wsc[kt, mt], wsc[kt, mt], psR[:], op=ALU.mult)

        conv2_mt(0)
        ar2_pre(0)           # flies while conv2(1) runs
        conv2_mt(1)
        ar2_pre(1)
        esB.close()

        esC = ExitStack()
        psC = esC.enter_context(tc.tile_pool(name="psC", bufs=4, space="PSUM"))

        # =====================================================================
        # Phase C: final fused pass  y = silu(a2*z2 + as*sc + (c2+cs))
        # pure DVE/ACT from the bf16 caches; mt0 runs while AllReduce#2(mt1)
        # is still in flight.
        # =====================================================================
        PF = 6

        def final_prefill(mt):
            tiles = []
            for b in range(NB):
                for rg in range(RG):
                    tiles.append((b, rg, psC.tile([128, TLEN], F32, tag="fz", name="fz", bufs=PF)))
            for b, rg, ps in tiles[:PF]:
                for kt in range(MT):
                    nc.tensor.matmul(
                        ps[:], w2[kt, mt],
                        t1[b, kt][:, rg * TLEN : (rg + 1) * TLEN],
                        start=(kt == 0), stop=False,
                    )
            return tiles

        def final_rest(mt, tiles):
            for i, (b, rg, ps) in enumerate(tiles):
                if i >= PF:
                    for kt in range(MT):
                        nc.tensor.matmul(
                            ps[:], w2[kt, mt],
                            t1[b, kt][:, rg * TLEN : (rg + 1) * TLEN],
                            start=(kt == 0), stop=False,
                        )
                for kt in range(MT):
                    nc.tensor.matmul(
                        ps[:], wsc[kt, mt], win(xm[b, kt], rg),
                        start=False, stop=(kt == MT - 1),
                    )
                stg = stage.tile([128, TLEN], F32, tag="stg", name="stg", bufs=4)
                nc.scalar.activation(
                    stg[:], ps[:], AF.Silu, bias=ccb[mt][:], scale=a2[mt][:]
                )
                yq3 = [nc.sync, nc.scalar, nc.gpsimd]
                yeng = yq3[(i + (0 if mt == 0 else 1)) % 3]
                yeng.dma_start(
                    y_d[b, mt * 128 : mt * 128 + 128, rg * RGR : (rg + 1) * RGR, :],
                    stg[:],
                )

        ar2_dma(0)
        ar2_dma(1)
        t0 = final_prefill(0)
        ar2_calc(0)
        final_rest(0, t0)
        t1f = final_prefill(1)
        ar2_calc(1)
        final_rest(1, t1f)
        esC.close()

    _split_sync_waits(nc)
    return nc


_NC = None


def _round_f32r(a):
    """Round-to-nearest-even to 11 mantissa bits (the PE's f32r format,
    verified bit-exact against the hardware cast DMA)."""
    x = np.ascontiguousarray(a, dtype=np.float32).view(np.uint32).astype(np.uint64)
    half = np.uint64(1 << 11)
    lowmask = np.uint64((1 << 12) - 1)
    lsb = np.uint64(1 << 12)
    exact_half = (x & lowmask) == half
    r = np.where(exact_half & ((x & lsb) == np.uint64(0)), x, x + half)
    r = r & np.uint64(0xFFFFF000)
    return r.astype(np.uint32).view(np.float32).reshape(a.shape)


def _prep_inputs(inputs):
    bf = ml_dtypes.bfloat16
    w_conv1 = inputs["w_conv1"]  # (C, C, 3, 3) OIHW
    w1t = np.ascontiguousarray(
        w_conv1.transpose(2, 3, 1, 0).reshape(9, C, C)
    ).astype(bf)  # [tap(kh*3+kw), cin, cout]
    w2t = np.ascontiguousarray(inputs["w_conv2"][:, :, 0, 0].T).astype(bf)
    wsct = np.ascontiguousarray(inputs["w_sc"][:, :, 0, 0].T).astype(bf)
    wdce_t = np.ascontiguousarray(inputs["w_dce"].T).astype(np.float32)
    wst = np.ascontiguousarray(inputs["w_shrink"].T).astype(np.float32)
    wet = np.ascontiguousarray(inputs["w_expand"].T).astype(np.float32)

    wch = inputs["w_ch"][:, 0]  # (C, 3, 3)
    # gath order: [S, rowE, row0, colE, col0, x(E,E), x(E,0), x(0,E), x(0,0)]
    chc = np.stack(
        [
            wch.sum((1, 2)),
            -wch[:, 0, :].sum(1),
            -wch[:, 2, :].sum(1),
            -wch[:, :, 0].sum(1),
            -wch[:, :, 2].sum(1),
            wch[:, 0, 0],
            wch[:, 0, 2],
            wch[:, 2, 0],
            wch[:, 2, 2],
        ],
        axis=1,
    ).astype(np.float32) / float(H * W)

    chv = np.stack(
        [
            inputs["b_dce"], inputs["g_bn1"], inputs["be_bn1"],
            inputs["g_bn2"], inputs["be_bn2"], inputs["g_bns"],
            inputs["be_bns"], inputs["b_expand"],
            inputs["g_bn2"], inputs["g_bns"], inputs["be_bn2"], inputs["be_bns"],
        ],
        axis=1,
    ).astype(np.float32)

    shared = {
        "w1t": w1t, "w2t": w2t, "wsct": wsct, "wdce_t": wdce_t,
        "wst": wst, "wet": wet, "chc": np.ascontiguousarray(chc),
        "chv": np.ascontiguousarray(chv),
        "bsh": inputs["b_shrink"].astype(np.float32),
    }
    in_maps = []
    for c in range(NCORES):
        m = dict(shared)
        xc = inputs["x"][c * NB : (c + 1) * NB]
        xp = np.zeros((NB, C, PLEN), np.float32)
        xp[:, :, : (H + 2) * PW].reshape(NB, C, H + 2, PW)[
            :, :, 1 : H + 1, 1 : W + 1
        ] = xc
        m["xp"] = xp.astype(bf)
        m["dce"] = np.ascontiguousarray(
            inputs["dce_output"][c * NB : (c + 1) * NB]
        ).astype(np.float32)
        in_maps.append(m)
    return in_maps


def kernel(**inputs):
    global _NC
    if _NC is None:
        _NC = _build()
    in_maps = _prep_inputs(inputs)
    res = run_bass_kernel_spmd(_NC, in_maps, list(range(NCORES)))
    return np.concatenate([res.results[c]["y"] for c in range(NCORES)], axis=0)


if __name__ == "__main__":
    nc = _build()
    print("build ok")
